# revision 1
# baseline (speedup 1.0000x reference)
"""Transformer block (nn_Block_49744311222996) on 8 TRN2 NeuronCores.

Sharding: core c = 2*b + g handles batch b (4 batches) and head-group g
(8 of 16 heads). Attention is computed head-parallel with unnormalized
exp + ones-column denominator (no max subtraction; logits are tiny).
The attention-output projection produces per-core partial sums which are
ReduceScatter'd (add) over core pairs so each core owns 512 of the 1024
query rows; LN2 + FFN then run fully locally. Output rows are gathered
on host.

Compute dtype: bf16 matmuls (fp32 PSUM accumulation), fp32 LN/softmax
pointwise. Weights are pre-cast/pre-sliced to bf16 on host.

Emission order pipelines V/K/Q projections with per-head-pair attention
so the TensorEngine stays dense (keeps the PE HAM clock at 2.4 GHz).
PSUM pools are phase-scoped: attention {mm:4, ops:4} and FFN {mm:4,
ff:4} each fit the 8 banks.
"""

import numpy as np
import ml_dtypes

import concourse.mybir as mybir
import concourse.tile as tile
from concourse import bacc
from concourse.bass_utils import run_bass_kernel_spmd

F32 = mybir.dt.float32
F32R = mybir.dt.float32r
BF16 = mybir.dt.bfloat16
AF = mybir.ActivationFunctionType
ALU = mybir.AluOpType

B, T, C = 4, 2048, 1024
H, HS = 16, 64
CUT = 1024  # query rows (last CUT positions)
P = 128
NT = T // P  # 16 t-tiles
NCt = C // P  # 8 c-tiles
GH = 8  # heads per core
EW = GH * HS  # 512: width of this core's head block
EPS = 1e-5
ATT_SCALE = float(C) ** -0.5  # reference scales by C**-0.5, not HS**-0.5
NF = 4 * C // P  # 32 f-tiles (FFN hidden 4096)
SROWS = 512  # seq rows owned per core after reduce-scatter


def _ln_group_stats(nc, pool, xts, width, eps_ap, act_square):
    """LN stats for a group of [128, width] fp32 APs (batched small ops).

    Returns (rstd, nmean) [128, len(xts)] fp32 tiles.
    """
    n = len(xts)
    s1 = pool.tile([P, n], F32, tag="s1")
    s2 = pool.tile([P, n], F32, tag="s2")
    for i, xt in enumerate(xts):
        nc.vector.reduce_sum(s1[:, i:i + 1], xt, axis=mybir.AxisListType.X)
        if act_square:
            sq = pool.tile([P, width], BF16, tag="sq")
            nc.scalar.activation(
                sq[:], xt, AF.Square, accum_out=s2[:, i:i + 1]
            )
        else:
            sq = pool.tile([P, width], BF16, tag="sq")
            nc.vector.tensor_mul(sq[:], xt, xt)
            nc.vector.reduce_sum(
                s2[:, i:i + 1], sq[:], axis=mybir.AxisListType.X
            )
    mean = pool.tile([P, n], F32, tag="mean")
    nc.vector.tensor_scalar_mul(mean[:], s1[:], 1.0 / width)
    ms = pool.tile([P, n], F32, tag="ms")
    nc.vector.tensor_mul(ms[:], mean[:], mean[:])
    var = pool.tile([P, n], F32, tag="var")
    nc.vector.scalar_tensor_tensor(
        out=var[:], in0=s2[:], scalar=1.0 / width, in1=ms[:],
        op0=ALU.mult, op1=ALU.subtract,
    )
    sd = pool.tile([P, n], F32, tag="sd")
    nc.scalar.activation(sd[:], var[:], AF.Sqrt, bias=eps_ap)
    rstd = pool.tile([P, n], F32, tag="rstd")
    nc.vector.reciprocal(rstd[:], sd[:])
    nmean = pool.tile([P, n], F32, tag="nmean")
    nc.vector.scalar_tensor_tensor(
        out=nmean[:], in0=mean[:], scalar=-1.0, in1=rstd[:],
        op0=ALU.mult, op1=ALU.mult,
    )
    return rstd, nmean


def build_nc():
    nc = bacc.Bacc(None, target_bir_lowering=False)

    # ---- DRAM parameters (per-core content prepared on host) ----
    x = nc.declare_dram_parameter("x", [T, C], F32, isOutput=False)
    xsl = nc.declare_dram_parameter("xslice", [SROWS, C], F32, isOutput=False)
    wq = nc.declare_dram_parameter("wq", [C, EW], BF16, isOutput=False)
    wk = nc.declare_dram_parameter("wk", [C, EW], BF16, isOutput=False)
    wv = nc.declare_dram_parameter("wv", [C, EW], BF16, isOutput=False)
    bq = nc.declare_dram_parameter("bq", [P, 4], F32, isOutput=False)
    bk = nc.declare_dram_parameter("bk", [P, 4], F32, isOutput=False)
    bv_bc = nc.declare_dram_parameter("bv_bc", [P, EW], F32, isOutput=False)
    wproj = nc.declare_dram_parameter("wproj", [EW, C], BF16, isOutput=False)
    bproj_bc = nc.declare_dram_parameter("bproj_bc", [P, C], F32, isOutput=False)
    ln1w = nc.declare_dram_parameter("ln1w", [P, NCt], F32, isOutput=False)
    ln1b = nc.declare_dram_parameter("ln1b", [P, NCt], F32, isOutput=False)
    ln2w = nc.declare_dram_parameter("ln2w", [P, NCt], F32, isOutput=False)
    ln2b = nc.declare_dram_parameter("ln2b", [P, NCt], F32, isOutput=False)
    lin1 = nc.declare_dram_parameter("lin1", [C, 4 * C], BF16, isOutput=False)
    blin1 = nc.declare_dram_parameter("blin1", [P, NF], F32, isOutput=False)
    lin2 = nc.declare_dram_parameter("lin2", [4 * C, C], BF16, isOutput=False)
    blin2_bc = nc.declare_dram_parameter("blin2_bc", [P, C], F32, isOutput=False)
    ident = nc.declare_dram_parameter("ident", [P, P], BF16, isOutput=False)
    masks = nc.declare_dram_parameter("masks", [P, 4 * 512], BF16, isOutput=False)
    out = nc.declare_dram_parameter("out", [SROWS, C], F32, isOutput=True)
    dbg = nc.declare_dram_parameter("dbg", [1, 1], F32, isOutput=True)

    x_tiles = x.rearrange("(n p) c -> n p c", p=P)
    out_tiles = out.rearrange("(n p) c -> n p c", p=P)

    with tile.TileContext(nc) as tc:
        with (
            tc.tile_pool(name="const", bufs=1) as const,
            tc.tile_pool(name="dram", bufs=1, space="DRAM") as dram,
            tc.tile_pool(name="stat", bufs=3) as stat,
            tc.tile_pool(name="wA", bufs=5) as wA,    # [128,1024] f32 loads
            tc.tile_pool(name="wB", bufs=3) as wB,    # [128,1024] bf16 h tiles
            tc.tile_pool(name="wC", bufs=4) as wC,    # [128,512] small tiles
        ):
            # ---- constants ----
            id_sb = const.tile([P, P], BF16)
            nc.sync.dma_start(id_sb[:], ident[:])
            mask_sb = const.tile([P, 4 * 512], BF16)
            nc.sync.dma_start(mask_sb[:], masks[:])
            bq_sb = const.tile([P, 4], F32)
            nc.sync.dma_start(bq_sb[:], bq[:])
            bk_sb = const.tile([P, 4], F32)
            nc.sync.dma_start(bk_sb[:], bk[:])
            bv_sb = const.tile([P, EW], F32)
            nc.sync.dma_start(bv_sb[:], bv_bc[:])
            ln1w_sb = const.tile([P, NCt], F32)
            nc.sync.dma_start(ln1w_sb[:], ln1w[:])
            ln1b_sb = const.tile([P, NCt], F32)
            nc.sync.dma_start(ln1b_sb[:], ln1b[:])
            ln2w_sb = const.tile([P, NCt], F32)
            nc.sync.dma_start(ln2w_sb[:], ln2w[:])
            ln2b_sb = const.tile([P, NCt], F32)
            nc.sync.dma_start(ln2b_sb[:], ln2b[:])
            bproj_sb = const.tile([P, C], F32)
            nc.sync.dma_start(bproj_sb[:], bproj_bc[:])
            blin1_sb = const.tile([P, NF], F32)
            nc.sync.dma_start(blin1_sb[:], blin1[:])
            blin2_sb = const.tile([P, C], F32)
            nc.sync.dma_start(blin2_sb[:], blin2_bc[:])
            ones_f = const.tile([1, HS], F32)
            nc.vector.memset(ones_f[:], 1.0)
            ones_sb = const.tile([1, HS], F32R)
            with nc.allow_low_precision(reason="f32r ones for bcast matmul"):
                nc.vector.reciprocal(ones_sb[:], ones_f[:])
            eps_sb = const.tile([P, 1], F32)
            nc.vector.memset(eps_sb[:], EPS)

            rs_in = dram.tile([CUT, C], BF16)
            rs_out = dram.tile([SROWS, C], BF16)

            with (
                tc.tile_pool(name="attA", bufs=1) as attA,
                tc.tile_pool(name="pP", bufs=2, space="PSUM") as pP,
                tc.tile_pool(name="pS", bufs=2, space="PSUM") as pS,
                tc.tile_pool(name="pO", bufs=2, space="PSUM") as pO,
            ):
                hT = attA.tile([P, NCt * T], BF16)     # h^T, c-tile j at [j*T,)
                kT = attA.tile([P, 4 * T], BF16)       # K^T, head-pair blocks
                qT = attA.tile([P, 4 * CUT], BF16)     # Q^T, head-pair blocks
                vaug = attA.tile([P, NT * 520], BF16)  # V+ones, t-tile blocks
                oT = attA.tile([P, 4 * CUT], BF16)     # o^T, e-tile blocks

                # ============ LN1 + transpose -> hT ============
                for grp in range(NT // 4):
                    xts = []
                    for i4 in range(4):
                        xt = wA.tile([P, C], F32, tag="xt")
                        nc.sync.dma_start(xt[:], x_tiles[grp * 4 + i4])
                        xts.append(xt)
                    rstd, nmean = _ln_group_stats(
                        nc, stat, [t[:] for t in xts], C, eps_sb[:],
                        act_square=True,
                    )
                    for i4 in range(4):
                        i = grp * 4 + i4
                        ht = wB.tile([P, C], BF16, tag="ht")
                        nc.scalar.activation(
                            ht[:], xts[i4][:], AF.Identity,
                            bias=nmean[:, i4:i4 + 1],
                            scale=rstd[:, i4:i4 + 1],
                        )
                        for j in range(NCt):
                            tp = pP.tile([P, P], BF16, tag="mm")
                            nc.tensor.transpose(
                                tp[:], ht[:, j * P:(j + 1) * P], id_sb[:]
                            )
                            nc.vector.tensor_scalar(
                                out=hT[:, j * T + i * P: j * T + (i + 1) * P],
                                in0=tp[:],
                                scalar1=ln1w_sb[:, j:j + 1],
                                scalar2=ln1b_sb[:, j:j + 1],
                                op0=ALU.mult, op1=ALU.add,
                            )

                # ---- weights for QKV ----
                with tc.tile_pool(name="wqkv", bufs=1) as wqkv:
                    wq_sb = wqkv.tile([P, NCt * EW], BF16)
                    wk_sb = wqkv.tile([P, NCt * EW], BF16)
                    wv_sb = wqkv.tile([P, NCt * EW], BF16)
                    wq_t = wq.rearrange("(n p) e -> n p e", p=P)
                    wk_t = wk.rearrange("(n p) e -> n p e", p=P)
                    wv_t = wv.rearrange("(n p) e -> n p e", p=P)
                    for j in range(NCt):
                        nc.sync.dma_start(wq_sb[:, j * EW:(j + 1) * EW], wq_t[j])
                        nc.sync.dma_start(wk_sb[:, j * EW:(j + 1) * EW], wk_t[j])
                        nc.sync.dma_start(wv_sb[:, j * EW:(j + 1) * EW], wv_t[j])

                    # ---- V (+ ones column) for all heads ----
                    nc.gpsimd.memset(vaug[:], 1.0)
                    for i in range(NT):
                        ps = pP.tile([P, 512], F32, tag="mm")
                        for j in range(NCt):
                            nc.tensor.matmul(
                                ps[:],
                                hT[:, j * T + i * P: j * T + (i + 1) * P],
                                wv_sb[:, j * EW:(j + 1) * EW],
                                start=(j == 0), stop=(j == NCt - 1),
                            )
                        va = vaug[:, i * 520:(i + 1) * 520].rearrange(
                            "p (h e) -> p h e", e=65
                        )
                        nc.vector.tensor_add(
                            va[:, :, 0:64],
                            ps[:].rearrange("p (h e) -> p h e", e=64),
                            bv_sb[:].rearrange("p (h e) -> p h e", e=64),
                        )

                    # ---- K,Q projections for all head-pairs ----
                    for hp in range(4):
                        for tch in range(4):  # K^T over all T
                            ps = pP.tile([P, 512], F32, tag="mm")
                            for j in range(NCt):
                                nc.tensor.matmul(
                                    ps[:],
                                    wk_sb[:, j * EW + hp * P:
                                          j * EW + (hp + 1) * P],
                                    hT[:, j * T + tch * 512:
                                       j * T + (tch + 1) * 512],
                                    start=(j == 0), stop=(j == NCt - 1),
                                )
                            nc.vector.tensor_scalar_add(
                                kT[:, hp * T + tch * 512:
                                   hp * T + (tch + 1) * 512],
                                ps[:], bk_sb[:, hp:hp + 1],
                            )
                        for sc in range(2):  # Q^T over last CUT
                            ps = pP.tile([P, 512], F32, tag="mm")
                            base = T - CUT
                            for j in range(NCt):
                                nc.tensor.matmul(
                                    ps[:],
                                    wq_sb[:, j * EW + hp * P:
                                          j * EW + (hp + 1) * P],
                                    hT[:, j * T + base + sc * 512:
                                       j * T + base + (sc + 1) * 512],
                                    start=(j == 0), stop=(j == NCt - 1),
                                )
                            nc.vector.tensor_scalar_add(
                                qT[:, hp * CUT + sc * 512:
                                   hp * CUT + (sc + 1) * 512],
                                ps[:], bq_sb[:, hp:hp + 1],
                            )

                # ---- attention: sc outer; paired S-tiles share one exp ----
                with tc.tile_pool(name="wpj", bufs=1) as wpj:
                    wp_sb = wpj.tile([P, 4 * C], BF16)
                    wp_t = wproj.rearrange("(n p) c -> n p c", p=P)
                    for et in range(4):
                        nc.sync.dma_start(wp_sb[:, et * C:(et + 1) * C], wp_t[et])
                    rs_in_t = rs_in.rearrange("(m p) u -> m p u", p=P)

                    def proj_mtile(m):
                        yst = wB.tile([P, C], BF16, tag="yst")
                        for nh in range(2):
                            ps = pP.tile([P, 512], F32, tag="mm")
                            for et in range(4):
                                nc.tensor.matmul(
                                    ps[:],
                                    oT[:, et * CUT + m * P:
                                       et * CUT + (m + 1) * P],
                                    wp_sb[:, et * C + nh * 512:
                                          et * C + (nh + 1) * 512],
                                    start=(et == 0), stop=(et == 3),
                                )
                            nc.scalar.activation(
                                yst[:, nh * 512:(nh + 1) * 512], ps[:], AF.Copy,
                            )
                        nc.sync.dma_start(rs_in_t[m], yst[:])

                    for sc in range(2):
                        n_vis = 12 + 4 * sc
                        for hp in range(4):
                            for r in range(2):
                                hh = 2 * hp + r
                                ops = pO.tile([65, 512], F32, tag="ops")
                                for tp2 in range(n_vis // 2):
                                    sps = pS.tile([P, 1024], F32, tag="sm")
                                    for half in range(2):
                                        tt = 2 * tp2 + half
                                        nc.tensor.matmul(
                                            sps[:, half * 512:(half + 1) * 512],
                                            kT[64 * r:64 * (r + 1),
                                               hp * T + tt * P:
                                               hp * T + (tt + 1) * P],
                                            qT[64 * r:64 * (r + 1),
                                               hp * CUT + sc * 512:
                                               hp * CUT + (sc + 1) * 512],
                                            start=True, stop=True,
                                        )
                                    pt = wC.tile([P, 1024], BF16, tag="pt")
                                    nc.scalar.activation(
                                        pt[:], sps[:], AF.Exp, scale=ATT_SCALE
                                    )
                                    for half in range(2):
                                        tt = 2 * tp2 + half
                                        off = tt * P - (T - CUT) - 512 * sc
                                        if off >= 0:
                                            kblk = off // P
                                            nc.vector.tensor_mul(
                                                pt[:, half * 512:
                                                   (half + 1) * 512],
                                                pt[:, half * 512:
                                                   (half + 1) * 512],
                                                mask_sb[:, kblk * 512:
                                                        (kblk + 1) * 512],
                                            )
                                        nc.tensor.matmul(
                                            ops[:],
                                            vaug[:, tt * 520 + hh * 65:
                                                 tt * 520 + (hh + 1) * 65],
                                            pt[:, half * 512:(half + 1) * 512],
                                            start=(tt == 0),
                                            stop=(tt == n_vis - 1),
                                        )
                                # rows 0:64 = o^T unnorm, row 64 = denom
                                rd = stat.tile([1, 512], F32R, tag="rd")
                                with nc.allow_low_precision(
                                    reason="f32r recip for attn denom bcast"
                                ):
                                    nc.vector.reciprocal(rd[:], ops[64:65, :])
                                rb = pP.tile([P, 512], F32, tag="mm")
                                nc.tensor.matmul(
                                    rb[0:64, :], ones_sb[:], rd[:],
                                    start=True, stop=True,
                                )
                                rbs = wC.tile([64, 512], F32, tag="rbs")
                                nc.vector.tensor_copy(rbs[:], rb[0:64, :])
                                nc.vector.tensor_mul(
                                    oT[64 * r:64 * (r + 1),
                                       hp * CUT + sc * 512:
                                       hp * CUT + (sc + 1) * 512],
                                    ops[0:64, :], rbs[:],
                                )
                        # proj for the s-blocks this sc chunk completed
                        for m in range(4 * sc, 4 * sc + 4):
                            proj_mtile(m)

                # ====== ReduceScatter (bf16, add) over pairs ======
                nc.gpsimd.collective_compute(
                    "ReduceScatter",
                    ALU.add,
                    replica_groups=[[0, 1], [2, 3], [4, 5], [6, 7]],
                    ins=[rs_in.opt()],
                    outs=[rs_out.opt()],
                )

            # ============ res + LN2 (attention pools freed) ============
            with (
                tc.tile_pool(name="ffA", bufs=1) as ffA,
                tc.tile_pool(name="pG", bufs=4, space="PSUM") as pG,
            ):
                # keep the PE HAM clock warm through the collective:
                # throwaway matmuls on const data, result shipped to dbg
                jps = pG.tile([P, 512], F32, tag="mm")
                for jj in range(120):
                    nc.tensor.matmul(
                        jps[:], mask_sb[:, 0:P], mask_sb[:, 0:512],
                        start=(jj == 0), stop=(jj == 119),
                    )
                jnk = stat.tile([1, 1], F32, tag="jnk")
                nc.vector.tensor_copy(jnk[:], jps[0:1, 0:1])
                nc.sync.dma_start(dbg[:], jnk[:])

                res = ffA.tile([P, 4 * C], F32)
                h2T = ffA.tile([P, NCt * SROWS], BF16)
                xs_t = xsl.rearrange("(n p) c -> n p c", p=P)
                rs_t = rs_out.rearrange("(n p) c -> n p c", p=P)
                for m in range(4):
                    xt = wA.tile([P, C], F32, tag="xt")
                    nc.sync.dma_start(xt[:], xs_t[m])
                    nc.vector.tensor_add(
                        res[:, m * C:(m + 1) * C], xt[:], bproj_sb[:]
                    )
                for m in range(4):
                    y2 = wB.tile([P, C], BF16, tag="yst")
                    nc.sync.dma_start(y2[:], rs_t[m])
                    rm = res[:, m * C:(m + 1) * C]
                    nc.vector.tensor_add(rm, rm, y2[:])
                    rstd, nmean = _ln_group_stats(
                        nc, stat, [rm], C, eps_sb[:], act_square=True
                    )
                    h2 = wB.tile([P, C], BF16, tag="ht")
                    nc.scalar.activation(
                        h2[:], rm, AF.Identity,
                        bias=nmean[:, 0:1], scale=rstd[:, 0:1],
                    )
                    for j in range(NCt):
                        tp = pG.tile([P, P], BF16, tag="mm")
                        nc.tensor.transpose(
                            tp[:], h2[:, j * P:(j + 1) * P], id_sb[:]
                        )
                        nc.vector.tensor_scalar(
                            out=h2T[:, j * SROWS + m * P:
                                    j * SROWS + (m + 1) * P],
                            in0=tp[:],
                            scalar1=ln2w_sb[:, j:j + 1],
                            scalar2=ln2b_sb[:, j:j + 1],
                            op0=ALU.mult, op1=ALU.add,
                        )
                    # fold lin2 bias into res for the final add
                    nc.vector.tensor_add(rm, rm, blin2_sb[:])

                # ============ FFN-1 (gelu tanh-approx) -> gT ============
                with (
                    tc.tile_pool(name="ffB", bufs=1) as ffB,
                    tc.tile_pool(name="ffW", bufs=2) as ffW,
                    tc.tile_pool(name="pF", bufs=1, space="PSUM") as pF,
                ):
                    lin1_sb = ffB.tile([P, NCt * 2 * C], BF16)
                    l1_t = lin1.rearrange("(n p) f -> n p f", p=P)
                    gT = ffB.tile([P, NF * SROWS], BF16)
                    for ft in range(NF):
                        if ft % (NF // 2) == 0:
                            half = ft // (NF // 2)
                            for j in range(NCt):
                                nc.sync.dma_start(
                                    lin1_sb[:, j * 2 * C:(j + 1) * 2 * C],
                                    l1_t[j][:, half * 2 * C:(half + 1) * 2 * C],
                                )
                        fl = ft % (NF // 2)
                        ps = pG.tile([P, 512], F32, tag="mm")
                        for j in range(NCt):
                            nc.tensor.matmul(
                                ps[:],
                                lin1_sb[:, j * 2 * C + fl * P:
                                        j * 2 * C + (fl + 1) * P],
                                h2T[:, j * SROWS:(j + 1) * SROWS],
                                start=(j == 0), stop=(j == NCt - 1),
                            )
                        gx = ffW.tile([P, SROWS], F32, tag="gx")
                        nc.scalar.activation(
                            gx[:], ps[:], AF.Identity,
                            bias=blin1_sb[:, ft:ft + 1],
                        )
                        u = ffW.tile([P, SROWS], F32, tag="gu")
                        nc.vector.tensor_mul(u[:], gx[:], gx[:])
                        nc.vector.tensor_scalar(
                            out=u[:], in0=u[:], scalar1=0.044715, scalar2=1.0,
                            op0=ALU.mult, op1=ALU.add,
                        )
                        nc.vector.tensor_mul(u[:], u[:], gx[:])
                        sg = ffW.tile([P, SROWS], F32, tag="gs")
                        nc.scalar.activation(
                            sg[:], u[:], AF.Sigmoid, scale=1.5957691216057308
                        )
                        nc.vector.tensor_mul(
                            gT[:, ft * SROWS:(ft + 1) * SROWS], gx[:], sg[:]
                        )

                    # ============ FFN-2 + residual -> out ============
                    l2_t = lin2.rearrange("(n p) c -> n p c", p=P)
                    for nh in range(2):
                        fps = []
                        for m in range(4):
                            fpt = pF.tile([P, 512], F32, tag=f"ff{m}")
                            fps.append(fpt)
                        for ft in range(NF):
                            l2 = ffW.tile([P, 512], BF16, tag="l2")
                            nc.sync.dma_start(
                                l2[:], l2_t[ft][:, nh * 512:(nh + 1) * 512]
                            )
                            for m in range(4):
                                nc.tensor.matmul(
                                    fps[m][:],
                                    gT[:, ft * SROWS + m * P:
                                       ft * SROWS + (m + 1) * P],
                                    l2[:],
                                    start=(ft == 0), stop=(ft == NF - 1),
                                )
                        for m in range(4):
                            o_sb = ffW.tile([P, 512], F32, tag="osb")
                            nc.vector.tensor_add(
                                o_sb[:], fps[m][:],
                                res[:, m * C + nh * 512:
                                    m * C + (nh + 1) * 512],
                            )
                            nc.sync.dma_start(
                                out_tiles[m][:, nh * 512:(nh + 1) * 512],
                                o_sb[:],
                            )

    nc.compile()
    return nc


_NC = None


def _get_nc():
    global _NC
    if _NC is None:
        _NC = build_nc()
    return _NC


def kernel(**inputs):
    nc = _get_nc()
    bf = ml_dtypes.bfloat16
    f32 = np.float32

    x = np.asarray(inputs["x"], f32)
    Wq = np.asarray(inputs["Wq"], f32)
    Wk = np.asarray(inputs["Wk"], f32)
    Wv = np.asarray(inputs["Wv"], f32)
    bq = np.asarray(inputs["bq"], f32)
    bk = np.asarray(inputs["bk"], f32)
    bv = np.asarray(inputs["bv"], f32)
    proj_w = np.asarray(inputs["proj_w"], f32)
    proj_b = np.asarray(inputs["proj_b"], f32)
    ln1_w = np.asarray(inputs["ln1_w"], f32)
    ln1_b = np.asarray(inputs["ln1_b"], f32)
    ln2_w = np.asarray(inputs["ln2_w"], f32)
    ln2_b = np.asarray(inputs["ln2_b"], f32)
    lin1_w = np.asarray(inputs["lin1_w"], f32)
    lin1_b = np.asarray(inputs["lin1_b"], f32)
    lin2_w = np.asarray(inputs["lin2_w"], f32)
    lin2_b = np.asarray(inputs["lin2_b"], f32)

    ident = np.eye(P, dtype=bf)
    tl = np.arange(P)[:, None]
    sl = np.arange(512)[None, :]
    masks = np.concatenate(
        [(sl >= tl + 128 * k).astype(bf) for k in range(4)], axis=1
    )
    ln1w_t = np.ascontiguousarray(ln1_w.reshape(NCt, P).T).astype(f32)
    ln1b_t = np.ascontiguousarray(ln1_b.reshape(NCt, P).T).astype(f32)
    ln2w_t = np.ascontiguousarray(ln2_w.reshape(NCt, P).T).astype(f32)
    ln2b_t = np.ascontiguousarray(ln2_b.reshape(NCt, P).T).astype(f32)
    lin1_bf = lin1_w.astype(bf)
    lin2_bf = lin2_w.astype(bf)
    blin1_t = np.ascontiguousarray(lin1_b.reshape(NF, P).T).astype(f32)
    blin2_bc = np.ascontiguousarray(np.broadcast_to(lin2_b, (P, C))).astype(f32)
    proj_w_bf = proj_w.astype(bf)
    bproj_bc = np.ascontiguousarray(np.broadcast_to(proj_b, (P, C))).astype(f32)

    in_maps = []
    for c in range(8):
        b, g = divmod(c, 2)
        hsl = slice(g * GH, (g + 1) * GH)
        wq_c = np.ascontiguousarray(
            Wq[hsl].transpose(1, 0, 2).reshape(C, EW)).astype(bf)
        wk_c = np.ascontiguousarray(
            Wk[hsl].transpose(1, 0, 2).reshape(C, EW)).astype(bf)
        wv_c = np.ascontiguousarray(
            Wv[hsl].transpose(1, 0, 2).reshape(C, EW)).astype(bf)
        bq_c = np.ascontiguousarray(bq[hsl].reshape(4, P).T).astype(f32)
        bk_c = np.ascontiguousarray(bk[hsl].reshape(4, P).T).astype(f32)
        bv_c = np.ascontiguousarray(
            np.broadcast_to(bv[hsl].reshape(EW), (P, EW))).astype(f32)
        wproj_c = np.ascontiguousarray(
            proj_w_bf[g * EW:(g + 1) * EW, :])
        xs = x[b, T - CUT + g * SROWS: T - CUT + (g + 1) * SROWS, :]
        in_maps.append({
            "x": np.ascontiguousarray(x[b]),
            "xslice": np.ascontiguousarray(xs),
            "wq": wq_c, "wk": wk_c, "wv": wv_c,
            "bq": bq_c, "bk": bk_c, "bv_bc": bv_c,
            "wproj": wproj_c, "bproj_bc": bproj_bc,
            "ln1w": ln1w_t, "ln1b": ln1b_t,
            "ln2w": ln2w_t, "ln2b": ln2b_t,
            "lin1": lin1_bf, "blin1": blin1_t,
            "lin2": lin2_bf, "blin2_bc": blin2_bc,
            "ident": ident, "masks": masks,
        })

    res = run_bass_kernel_spmd(nc, in_maps, core_ids=list(range(8)))
    out_full = np.empty((B, CUT, C), f32)
    for c in range(8):
        b, g = divmod(c, 2)
        out_full[b, g * SROWS:(g + 1) * SROWS, :] = res.results[c]["out"]
    return out_full



# revision 5
# speedup vs baseline: 1.0770x; 1.0770x over previous
"""Transformer block (nn_Block_49744311222996) on 8 TRN2 NeuronCores.

Sharding: core c = 2*b + g handles batch b (4 batches); the 1024 query
rows are split between the two cores of a batch in 64-row interleaved
blocks (core g takes global q-blocks {2j+g}), which makes the causal
visible-tile structure identical on every core (n_vis = 9..16) and the
exp/softmax volume perfectly balanced. Each core computes K/V for ALL
16 heads over the full T=2048 (small duplicated matmul work) so that
attention + output projection + FFN for its 512 rows are fully local:
NO collectives at all.

Attention is head-parallel with unnormalized exp + ones-column
denominator (logits are tiny, no max subtraction). Score matmuls for a
head-pair are packed 2x onto the PE array via tile_position row tiling
(K=64 each). QKV projections are emitted interleaved with per-head-pair
attention so TensorE work overlaps the ScalarE exp stream.

Algebraic folds (exact): LN1 gamma/beta folded into Wq/Wk/Wv (+ bias
terms), LN2 gamma/beta folded into lin1, K-projection bias dropped
(softmax shift invariance), proj bias applied via a K=1 matmul row,
lin2 bias folded into the residual after LN2 reads it.

Compute dtype: bf16 matmuls (fp32 PSUM accumulation), fp32 LN/softmax
pointwise, single-instruction Gelu on ScalarE.
"""

import numpy as np
import ml_dtypes

import concourse.mybir as mybir
import concourse.tile as tile
from concourse import bacc
from concourse.bass_utils import run_bass_kernel_spmd

F32 = mybir.dt.float32
F32R = mybir.dt.float32r
BF16 = mybir.dt.bfloat16
AF = mybir.ActivationFunctionType
ALU = mybir.AluOpType

B, T, C = 4, 2048, 1024
H, HS = 16, 64
CUT = 1024
P = 128
NT = T // P          # 16 t-tiles
NCt = C // P         # 8 c-tiles
NHP = 8              # head pairs (16 heads)
EPS = 1e-5
ATT_SCALE = float(C) ** -0.5
NF = 4 * C // P      # 32 f-tiles
SR = 512             # q rows per core
NM = SR // P         # 4 q m-tiles
VW = H * 65          # vaug width per t-tile (16 heads x (64+ones))


def _ln_group_stats(nc, pool, xts, width, eps_ap):
    """LN stats for a group of [128, width] fp32 APs.

    Returns (rstd, nmean) [128, len(xts)] fp32 tiles.
    """
    n = len(xts)
    s1 = pool.tile([P, n], F32, tag="s1")
    s2 = pool.tile([P, n], F32, tag="s2")
    for i, xt in enumerate(xts):
        nc.vector.reduce_sum(s1[:, i:i + 1], xt, axis=mybir.AxisListType.X)
        sq = pool.tile([P, width], BF16, tag="sq")
        nc.scalar.activation(sq[:], xt, AF.Square, accum_out=s2[:, i:i + 1])
    mean = pool.tile([P, n], F32, tag="mean")
    nc.vector.tensor_scalar_mul(mean[:], s1[:], 1.0 / width)
    ms = pool.tile([P, n], F32, tag="ms")
    nc.vector.tensor_mul(ms[:], mean[:], mean[:])
    var = pool.tile([P, n], F32, tag="var")
    nc.vector.scalar_tensor_tensor(
        out=var[:], in0=s2[:], scalar=1.0 / width, in1=ms[:],
        op0=ALU.mult, op1=ALU.subtract,
    )
    sd = pool.tile([P, n], F32, tag="sd")
    nc.scalar.activation(sd[:], var[:], AF.Sqrt, bias=eps_ap)
    rstd = pool.tile([P, n], F32, tag="rstd")
    nc.vector.reciprocal(rstd[:], sd[:])
    nmean = pool.tile([P, n], F32, tag="nmean")
    nc.vector.scalar_tensor_tensor(
        out=nmean[:], in0=mean[:], scalar=-1.0, in1=rstd[:],
        op0=ALU.mult, op1=ALU.mult,
    )
    return rstd, nmean


def build_nc():
    nc = bacc.Bacc(None, target_bir_lowering=False)

    x = nc.declare_dram_parameter("x", [T, C], F32, isOutput=False)
    xq = nc.declare_dram_parameter("xq", [SR, C], F32, isOutput=False)
    wq = nc.declare_dram_parameter("wq", [C, C], BF16, isOutput=False)
    wk = nc.declare_dram_parameter("wk", [C, C], BF16, isOutput=False)
    wv = nc.declare_dram_parameter("wv", [C, C], BF16, isOutput=False)
    bq = nc.declare_dram_parameter("bq", [P, NHP], F32, isOutput=False)
    bv_bc = nc.declare_dram_parameter("bv_bc", [P, C], F32, isOutput=False)
    wproj = nc.declare_dram_parameter("wproj", [C, C], BF16, isOutput=False)
    projb = nc.declare_dram_parameter("projb", [1, C], BF16, isOutput=False)
    lin1 = nc.declare_dram_parameter("lin1", [C, 4 * C], BF16, isOutput=False)
    blin1 = nc.declare_dram_parameter("blin1", [P, NF], F32, isOutput=False)
    lin2 = nc.declare_dram_parameter("lin2", [4 * C, C], BF16, isOutput=False)
    blin2_bc = nc.declare_dram_parameter("blin2_bc", [P, C], F32,
                                         isOutput=False)
    ident = nc.declare_dram_parameter("ident", [P, P], BF16, isOutput=False)
    masks = nc.declare_dram_parameter("masks", [P, 1024], BF16, isOutput=False)
    out = nc.declare_dram_parameter("out", [SR, C], F32, isOutput=True)

    x_tiles = x.rearrange("(n p) c -> n p c", p=P)
    xq_tiles = xq.rearrange("(n p) c -> n p c", p=P)
    out_tiles = out.rearrange("(n p) c -> n p c", p=P)

    with tile.TileContext(nc) as tc:
        with (
            tc.tile_pool(name="const", bufs=1) as const,
            tc.tile_pool(name="stat", bufs=3) as stat,
            tc.tile_pool(name="wB", bufs=3) as wB,    # [128,1024] bf16 h tiles
        ):
            id_sb = const.tile([P, P], BF16)
            nc.sync.dma_start(id_sb[:], ident[:])
            mask_sb = const.tile([P, 1024], BF16)
            nc.sync.dma_start(mask_sb[:], masks[:])
            bq_sb = const.tile([P, NHP], F32)
            nc.sync.dma_start(bq_sb[:], bq[:])
            bv_sb = const.tile([P, C], F32)
            nc.sync.dma_start(bv_sb[:], bv_bc[:])
            projb_sb = const.tile([1, C], BF16)
            nc.sync.dma_start(projb_sb[:], projb[:])
            blin1_sb = const.tile([P, NF], F32)
            nc.sync.dma_start(blin1_sb[:], blin1[:])
            blin2_sb = const.tile([P, C], F32)
            nc.sync.dma_start(blin2_sb[:], blin2_bc[:])
            ones_f = const.tile([1, HS], F32)
            nc.vector.memset(ones_f[:], 1.0)
            ones_sb = const.tile([1, HS], F32R)
            with nc.allow_low_precision(reason="f32r ones for bcast matmul"):
                nc.vector.reciprocal(ones_sb[:], ones_f[:])
            onescol = const.tile([1, P], BF16)
            nc.vector.memset(onescol[:], 1.0)
            eps_sb = const.tile([P, 1], F32)
            nc.vector.memset(eps_sb[:], EPS)

            # persistent across the whole kernel
            res = const.tile([P, NM * C], F32)      # xq, then residual
            oT = const.tile([P, NHP * SR], BF16)    # per-pair o^T blocks
            wp_sb = const.tile([P, NHP * C], BF16)  # proj weights
            wp_t = wproj.rearrange("(n p) c -> n p c", p=P)
            for hp in range(NHP):
                nc.sync.dma_start(wp_sb[:, hp * C:(hp + 1) * C], wp_t[hp])

            with tc.tile_pool(name="abig", bufs=1) as abig:
                hT = abig.tile([P, NCt * T], BF16)    # h^T (c-tile j at j*T)
                hqT = abig.tile([P, NCt * SR], BF16)  # hq^T (my q rows)
                vaug = abig.tile([P, NT * VW], BF16)  # V+ones per t-tile

                # ones columns of vaug (col 64 of each head block)
                va4 = vaug[:].rearrange("p (t h e) -> p t h e", h=H, e=65)
                nc.vector.memset(va4[:, :, :, 64:65], 1.0)

                # ============ hq: LN1 on my q rows -> hqT ============
                with tc.tile_pool(name="pT", bufs=4, space="PSUM") as pT:
                    for m in range(NM):
                        nc.sync.dma_start(
                            res[:, m * C:(m + 1) * C], xq_tiles[m]
                        )
                    rstd, nmean = _ln_group_stats(
                        nc, stat,
                        [res[:, m * C:(m + 1) * C] for m in range(NM)],
                        C, eps_sb[:],
                    )
                    for m in range(NM):
                        hqm = wB.tile([P, C], BF16, tag="ht")
                        nc.scalar.activation(
                            hqm[:], res[:, m * C:(m + 1) * C], AF.Identity,
                            bias=nmean[:, m:m + 1], scale=rstd[:, m:m + 1],
                        )
                        for j in range(NCt):
                            tp = pT.tile([P, P], BF16, tag="tp")
                            nc.tensor.transpose(
                                tp[:], hqm[:, j * P:(j + 1) * P], id_sb[:]
                            )
                            nc.vector.tensor_copy(
                                hqT[:, j * SR + m * P: j * SR + (m + 1) * P],
                                tp[:],
                            )

                    # ======= LN1 over full T -> hT; V proj per tile =======
                    with (
                        tc.tile_pool(name="wv_p", bufs=1) as wv_p,
                        tc.tile_pool(name="wA", bufs=5) as wA,
                        tc.tile_pool(name="pV", bufs=2, space="PSUM") as pV,
                    ):
                        wv_sb = wv_p.tile([P, NCt * C], BF16)
                        wv_t = wv.rearrange("(n p) e -> n p e", p=P)
                        for j in range(NCt):
                            nc.sync.dma_start(
                                wv_sb[:, j * C:(j + 1) * C], wv_t[j]
                            )

                        for grp in range(NT // 4):
                            xts = []
                            for i4 in range(4):
                                xt = wA.tile([P, C], F32, tag="xt")
                                nc.sync.dma_start(
                                    xt[:], x_tiles[grp * 4 + i4]
                                )
                                xts.append(xt)
                            rstd, nmean = _ln_group_stats(
                                nc, stat, [t[:] for t in xts], C, eps_sb[:],
                            )
                            for i4 in range(4):
                                i = grp * 4 + i4
                                ht = wB.tile([P, C], BF16, tag="ht")
                                nc.scalar.activation(
                                    ht[:], xts[i4][:], AF.Identity,
                                    bias=nmean[:, i4:i4 + 1],
                                    scale=rstd[:, i4:i4 + 1],
                                )
                                for j in range(NCt):
                                    tp = pT.tile([P, P], BF16, tag="tp")
                                    nc.tensor.transpose(
                                        tp[:], ht[:, j * P:(j + 1) * P],
                                        id_sb[:]
                                    )
                                    nc.vector.tensor_copy(
                                        hT[:, j * T + i * P:
                                           j * T + (i + 1) * P],
                                        tp[:],
                                    )
                                # V projection for this t-tile (16 heads)
                                for eh in range(2):
                                    ps = pV.tile([P, 512], F32, tag="vps")
                                    for j in range(NCt):
                                        nc.tensor.matmul(
                                            ps[:],
                                            hT[:, j * T + i * P:
                                               j * T + (i + 1) * P],
                                            wv_sb[:, j * C + eh * 512:
                                                  j * C + (eh + 1) * 512],
                                            start=(j == 0),
                                            stop=(j == NCt - 1),
                                        )
                                    va = vaug[:, i * VW + eh * 8 * 65:
                                              i * VW + (eh + 1) * 8 * 65]
                                    va = va.rearrange(
                                        "p (h e) -> p h e", e=65
                                    )
                                    nc.vector.tensor_add(
                                        va[:, :, 0:64],
                                        ps[:].rearrange(
                                            "p (h e) -> p h e", e=64
                                        ),
                                        bv_sb[:, eh * 512:(eh + 1) * 512]
                                        .rearrange("p (h e) -> p h e", e=64),
                                    )

                # ============ attention: per head-pair ============
                wk_t = wk.rearrange("(n p) e -> n p e", p=P)
                wq_t = wq.rearrange("(n p) e -> n p e", p=P)
                with (
                    tc.tile_pool(name="wkq", bufs=2) as wkq,
                    tc.tile_pool(name="kTp", bufs=2) as kTp,
                    tc.tile_pool(name="qTp", bufs=2) as qTp,
                    tc.tile_pool(name="ptp", bufs=3) as ptp,
                    tc.tile_pool(name="pS", bufs=2, space="PSUM") as pS,
                    tc.tile_pool(name="pO", bufs=2, space="PSUM") as pO,
                    tc.tile_pool(name="pM", bufs=2, space="PSUM") as pM,
                ):
                    for hp in range(NHP):
                        wk_sb = wkq.tile([P, NCt * P], BF16, tag="wk")
                        wq_sb = wkq.tile([P, NCt * P], BF16, tag="wq")
                        for j in range(NCt):
                            nc.sync.dma_start(
                                wk_sb[:, j * P:(j + 1) * P],
                                wk_t[j][:, hp * P:(hp + 1) * P],
                            )
                            nc.sync.dma_start(
                                wq_sb[:, j * P:(j + 1) * P],
                                wq_t[j][:, hp * P:(hp + 1) * P],
                            )
                        # K^T for this pair over all T
                        kT = kTp.tile([P, T], BF16, tag="kT")
                        for tch in range(4):
                            ps = pM.tile([P, 512], F32, tag="mm")
                            for j in range(NCt):
                                nc.tensor.matmul(
                                    ps[:],
                                    wk_sb[:, j * P:(j + 1) * P],
                                    hT[:, j * T + tch * 512:
                                       j * T + (tch + 1) * 512],
                                    start=(j == 0), stop=(j == NCt - 1),
                                )
                            nc.vector.tensor_copy(
                                kT[:, tch * 512:(tch + 1) * 512], ps[:]
                            )
                        # Q^T for this pair over my 512 rows
                        qT = qTp.tile([P, SR], BF16, tag="qT")
                        ps = pM.tile([P, 512], F32, tag="mm")
                        for j in range(NCt):
                            nc.tensor.matmul(
                                ps[:],
                                wq_sb[:, j * P:(j + 1) * P],
                                hqT[:, j * SR:(j + 1) * SR],
                                start=(j == 0), stop=(j == NCt - 1),
                            )
                        nc.vector.tensor_scalar_add(
                            qT[:], ps[:], bq_sb[:, hp:hp + 1]
                        )

                        # ops accumulators (rows 0:64 o^T, row 64 denom)
                        opsA = pO.tile([65, SR], F32, tag="ops")
                        opsB = pO.tile([65, SR], F32, tag="ops")
                        for ch in range(2):
                            n_vis = 12 + 4 * ch
                            for g in range(n_vis // 4):
                                sA = pS.tile([P, 1024], F32, tag="sc")
                                sB = pS.tile([P, 1024], F32, tag="sc")
                                for t4 in range(4):
                                    tt = g * 4 + t4
                                    nc.tensor.matmul(
                                        sA[:, t4 * 256:(t4 + 1) * 256],
                                        kT[0:64, tt * P:(tt + 1) * P],
                                        qT[0:64, ch * 256:(ch + 1) * 256],
                                        start=True, stop=True,
                                    )
                                    nc.tensor.matmul(
                                        sB[:, t4 * 256:(t4 + 1) * 256],
                                        kT[64:128, tt * P:(tt + 1) * P],
                                        qT[64:128, ch * 256:(ch + 1) * 256],
                                        start=True, stop=True,
                                    )
                                ptA = ptp.tile([P, 1024], BF16, tag="pt")
                                ptB = ptp.tile([P, 1024], BF16, tag="pt")
                                nc.scalar.activation(
                                    ptA[:], sA[:], AF.Exp, scale=ATT_SCALE
                                )
                                nc.scalar.activation(
                                    ptB[:], sB[:], AF.Exp, scale=ATT_SCALE
                                )
                                if g == 2 + ch:  # boundary group: masks
                                    nc.vector.tensor_mul(
                                        ptA[:], ptA[:], mask_sb[:]
                                    )
                                    nc.vector.tensor_mul(
                                        ptB[:], ptB[:], mask_sb[:]
                                    )
                                for t4 in range(4):
                                    tt = g * 4 + t4
                                    nc.tensor.matmul(
                                        opsA[:, ch * 256:(ch + 1) * 256],
                                        vaug[:, tt * VW + 2 * hp * 65:
                                             tt * VW + 2 * hp * 65 + 65],
                                        ptA[:, t4 * 256:(t4 + 1) * 256],
                                        start=(tt == 0),
                                        stop=(tt == n_vis - 1),
                                    )
                                    nc.tensor.matmul(
                                        opsB[:, ch * 256:(ch + 1) * 256],
                                        vaug[:, tt * VW + (2 * hp + 1) * 65:
                                             tt * VW + (2 * hp + 1) * 65
                                             + 65],
                                        ptB[:, t4 * 256:(t4 + 1) * 256],
                                        start=(tt == 0),
                                        stop=(tt == n_vis - 1),
                                    )
                        # normalize: oT[...] = ops[0:64] / ops[64]
                        for r, ops in ((0, opsA), (1, opsB)):
                            rd = stat.tile([1, SR], F32R, tag="rd")
                            with nc.allow_low_precision(
                                reason="f32r recip for attn denom bcast"
                            ):
                                nc.vector.reciprocal(rd[:], ops[64:65, :])
                            rb = pM.tile([P, 512], F32, tag="mm")
                            nc.tensor.matmul(
                                rb[0:64, :], ones_sb[:], rd[:],
                                start=True, stop=True,
                            )
                            rbs = stat.tile([64, SR], F32, tag="rbs")
                            nc.vector.tensor_copy(rbs[:], rb[0:64, :])
                            nc.vector.tensor_mul(
                                oT[64 * r:64 * (r + 1),
                                   hp * SR:(hp + 1) * SR],
                                ops[0:64, :], rbs[:],
                            )

            # ============ proj + residual -> res; LN2 -> h2T; FFN =======
            with tc.tile_pool(name="post", bufs=1) as post:
                h2T = post.tile([P, NCt * SR], BF16)
                gT = post.tile([P, NF * SR], BF16)
                with (
                    tc.tile_pool(name="pP2", bufs=3, space="PSUM") as pP2,
                    tc.tile_pool(name="pT2", bufs=4, space="PSUM") as pT2,
                ):
                    for m in range(NM):
                        for nh in range(2):
                            ps = pP2.tile([P, 512], F32, tag="mm")
                            for hp in range(NHP):
                                nc.tensor.matmul(
                                    ps[:],
                                    oT[:, hp * SR + m * P:
                                       hp * SR + (m + 1) * P],
                                    wp_sb[:, hp * C + nh * 512:
                                          hp * C + (nh + 1) * 512],
                                    start=(hp == 0), stop=False,
                                )
                            # proj bias via K=1 matmul (ones x bias row)
                            nc.tensor.matmul(
                                ps[:], onescol[:],
                                projb_sb[:, nh * 512:(nh + 1) * 512],
                                start=False, stop=True,
                            )
                            rm = res[:, m * C + nh * 512:
                                     m * C + (nh + 1) * 512]
                            nc.vector.tensor_add(rm, rm, ps[:])
                    rstd, nmean = _ln_group_stats(
                        nc, stat,
                        [res[:, m * C:(m + 1) * C] for m in range(NM)],
                        C, eps_sb[:],
                    )
                    for m in range(NM):
                        h2 = wB.tile([P, C], BF16, tag="ht")
                        nc.scalar.activation(
                            h2[:], res[:, m * C:(m + 1) * C], AF.Identity,
                            bias=nmean[:, m:m + 1], scale=rstd[:, m:m + 1],
                        )
                        for j in range(NCt):
                            tp = pT2.tile([P, P], BF16, tag="tp")
                            nc.tensor.transpose(
                                tp[:], h2[:, j * P:(j + 1) * P], id_sb[:]
                            )
                            nc.vector.tensor_copy(
                                h2T[:, j * SR + m * P: j * SR + (m + 1) * P],
                                tp[:],
                            )
                        # fold lin2 bias into res for the final add
                        nc.vector.tensor_add(
                            res[:, m * C:(m + 1) * C],
                            res[:, m * C:(m + 1) * C], blin2_sb[:],
                        )

                # ============ FFN1 (gelu) -> gT ============
                l1_t = lin1.rearrange("(n p) f -> n p f", p=P)
                with (
                    tc.tile_pool(name="ffB", bufs=2) as ffB,
                    tc.tile_pool(name="pG", bufs=2, space="PSUM") as pG,
                ):
                    for quar in range(4):
                        l1h = ffB.tile([P, NCt * C], BF16, tag="l1")
                        for j in range(NCt):
                            nc.sync.dma_start(
                                l1h[:, j * C:(j + 1) * C],
                                l1_t[j][:, quar * C:(quar + 1) * C],
                            )
                        for fl in range(NF // 4):
                            ft = quar * (NF // 4) + fl
                            ps = pG.tile([P, 512], F32, tag="mm")
                            for j in range(NCt):
                                nc.tensor.matmul(
                                    ps[:],
                                    l1h[:, j * C + fl * P:
                                        j * C + (fl + 1) * P],
                                    h2T[:, j * SR:(j + 1) * SR],
                                    start=(j == 0), stop=(j == NCt - 1),
                                )
                            nc.scalar.activation(
                                gT[:, ft * SR:(ft + 1) * SR], ps[:], AF.Gelu,
                                bias=blin1_sb[:, ft:ft + 1],
                            )

                # ============ FFN2 + residual -> out ============
                l2_t = lin2.rearrange("(n p) c -> n p c", p=P)
                with (
                    tc.tile_pool(name="ffW", bufs=3) as ffW,
                    tc.tile_pool(name="pF", bufs=1, space="PSUM") as pF,
                ):
                    for nh in range(2):
                        fps = []
                        for m in range(NM):
                            fpt = pF.tile([P, 512], F32, tag=f"ff{m}")
                            fps.append(fpt)
                        for ft in range(NF):
                            l2 = ffW.tile([P, 512], BF16, tag="l2")
                            nc.sync.dma_start(
                                l2[:], l2_t[ft][:, nh * 512:(nh + 1) * 512]
                            )
                            for m in range(NM):
                                nc.tensor.matmul(
                                    fps[m][:],
                                    gT[:, ft * SR + m * P:
                                       ft * SR + (m + 1) * P],
                                    l2[:],
                                    start=(ft == 0), stop=(ft == NF - 1),
                                )
                        for m in range(NM):
                            o_sb = ffW.tile([P, 512], F32, tag="osb")
                            nc.vector.tensor_add(
                                o_sb[:], fps[m][:],
                                res[:, m * C + nh * 512:
                                    m * C + (nh + 1) * 512],
                            )
                            nc.sync.dma_start(
                                out_tiles[m][:, nh * 512:(nh + 1) * 512],
                                o_sb[:],
                            )

    nc.compile()
    return nc


_NC = None


def _get_nc():
    global _NC
    if _NC is None:
        _NC = build_nc()
    return _NC


def kernel(**inputs):
    nc = _get_nc()
    bf = ml_dtypes.bfloat16
    f32 = np.float32

    x = np.asarray(inputs["x"], f32)
    Wq = np.asarray(inputs["Wq"], f32)
    Wk = np.asarray(inputs["Wk"], f32)
    Wv = np.asarray(inputs["Wv"], f32)
    bq = np.asarray(inputs["bq"], f32)
    bv = np.asarray(inputs["bv"], f32)
    proj_w = np.asarray(inputs["proj_w"], f32)
    proj_b = np.asarray(inputs["proj_b"], f32)
    ln1_w = np.asarray(inputs["ln1_w"], f32)
    ln1_b = np.asarray(inputs["ln1_b"], f32)
    ln2_w = np.asarray(inputs["ln2_w"], f32)
    ln2_b = np.asarray(inputs["ln2_b"], f32)
    lin1_w = np.asarray(inputs["lin1_w"], f32)
    lin1_b = np.asarray(inputs["lin1_b"], f32)
    lin2_w = np.asarray(inputs["lin2_w"], f32)
    lin2_b = np.asarray(inputs["lin2_b"], f32)

    # LN1 gamma folded into QKV weights; beta folded into biases.
    Wq_f = Wq * ln1_w[None, :, None]              # [H, C, HS]
    Wk_f = Wk * ln1_w[None, :, None]
    Wv_f = Wv * ln1_w[None, :, None]
    bq_eff = bq + np.einsum("c,hcd->hd", ln1_b, Wq)   # [H, HS]
    bv_eff = bv + np.einsum("c,hcd->hd", ln1_b, Wv)
    # (bk dropped: softmax shift invariance)
    wq_full = np.ascontiguousarray(
        Wq_f.transpose(1, 0, 2).reshape(C, C)).astype(bf)
    wk_full = np.ascontiguousarray(
        Wk_f.transpose(1, 0, 2).reshape(C, C)).astype(bf)
    wv_full = np.ascontiguousarray(
        Wv_f.transpose(1, 0, 2).reshape(C, C)).astype(bf)
    bq_t = np.ascontiguousarray(bq_eff.reshape(NHP, P).T).astype(f32)
    bv_bc = np.ascontiguousarray(
        np.broadcast_to(bv_eff.reshape(C), (P, C))).astype(f32)

    # LN2 gamma folded into lin1; beta into its bias.
    lin1_f = (lin1_w * ln2_w[:, None]).astype(bf)
    blin1_eff = lin1_b + ln2_b @ lin1_w
    blin1_t = np.ascontiguousarray(blin1_eff.reshape(NF, P).T).astype(f32)
    lin2_bf = lin2_w.astype(bf)
    blin2_bc = np.ascontiguousarray(
        np.broadcast_to(lin2_b, (P, C))).astype(f32)

    proj_bf = proj_w.astype(bf)
    projb_row = proj_b.reshape(1, C).astype(bf)
    ident = np.eye(P, dtype=bf)

    in_maps = []
    for c in range(8):
        b, g = divmod(c, 2)
        # my q rows: 64-row blocks {2j+g}, local row 64j+r
        j = np.arange(NCt)
        r = np.arange(64)
        rows = (T - CUT) + 64 * (2 * j[:, None] + g) + r[None, :]
        rows = rows.reshape(-1)
        xq_c = np.ascontiguousarray(x[b][rows]).astype(f32)
        # masks: [128, 4*256]: block i, col 64*jj+rr:
        #   visible iff toff <= 128*jj + 64*g + rr - 128*i
        toff = np.arange(P)[:, None]
        i_b = np.arange(4)[:, None, None]
        jj = np.arange(4)[None, :, None]
        rr = np.arange(64)[None, None, :]
        thr = (128 * jj + 64 * g + rr - 128 * i_b).reshape(1, 1024)
        mask_c = (toff <= thr).astype(bf)
        in_maps.append({
            "x": np.ascontiguousarray(x[b]),
            "xq": xq_c,
            "wq": wq_full, "wk": wk_full, "wv": wv_full,
            "bq": bq_t, "bv_bc": bv_bc,
            "wproj": proj_bf, "projb": projb_row,
            "lin1": lin1_f, "blin1": blin1_t,
            "lin2": lin2_bf, "blin2_bc": blin2_bc,
            "ident": ident, "masks": np.ascontiguousarray(mask_c),
        })

    resl = run_bass_kernel_spmd(nc, in_maps, core_ids=list(range(8)))
    out_full = np.empty((B, CUT, C), f32)
    jj = np.arange(NCt)
    rr = np.arange(64)
    for c in range(8):
        b, g = divmod(c, 2)
        rows = (64 * (2 * jj[:, None] + g) + rr[None, :]).reshape(-1)
        out_full[b, rows, :] = resl.results[c]["out"]
    return out_full


# revision 15
# speedup vs baseline: 1.2177x; 1.1307x over previous
"""Transformer block (nn_Block_49744311222996) on 8 TRN2 NeuronCores.

Sharding: core c = 2*b + g handles batch b (4 batches); the 1024 query
rows are split between the two cores of a batch in 64-row interleaved
blocks (core g takes global q-blocks {2j+g}), which makes the causal
visible-tile structure identical on every core (n_vis = 9..16) and the
exp/softmax volume perfectly balanced. Each core computes K/V for ALL
16 heads over the full T=2048 (small duplicated matmul work) so that
attention + output projection + FFN for its 512 rows are fully local:
NO collectives at all.

Attention is head-parallel with unnormalized exp + ones-column
denominator (logits are tiny, no max subtraction). Score matmuls for a
head-pair are packed 2x onto the PE array via tile_position row tiling
(K=64 each). K/Q projection chains for pair hp+1 are emitted BETWEEN
the score groups of pair hp (the PE runs in emission order, so this
fills the exp-wait bubbles), and attV for group g is emitted after the
scores of group g+1 (software pipelining).

Algebraic folds (exact): LN1 gamma/beta folded into Wq/Wk/Wv (+ bias
terms), LN2 gamma/beta folded into lin1, K-projection bias dropped
(softmax shift invariance), proj/V biases applied via K=1 matmul rows.

Compute dtype: bf16 matmuls (fp32 PSUM accumulation), fp32 LN/softmax
pointwise, single-instruction Gelu on ScalarE, LN stats via bn_stats.
"""

import numpy as np
import ml_dtypes

import concourse.mybir as mybir
import concourse.tile as tile
from concourse import bacc
from concourse.bass_utils import run_bass_kernel_spmd

F32 = mybir.dt.float32
BF16 = mybir.dt.bfloat16
AF = mybir.ActivationFunctionType
ALU = mybir.AluOpType

B, T, C = 4, 2048, 1024
H, HS = 16, 64
CUT = 1024
P = 128
NT = T // P          # 16 t-tiles
NCt = C // P         # 8 c-tiles
NHP = 8              # head pairs (16 heads)
EPS = 1e-5
ATT_SCALE = float(C) ** -0.5
NF = 4 * C // P      # 32 f-tiles
SR = 512             # q rows per core
NM = SR // P         # 4 q m-tiles
VW = H * 65          # vaug width per t-tile (16 heads x (64+ones))


def _ln_group_stats(nc, pool, xts, eps_ap):
    """LN stats for a group of [128, 1024] fp32 APs via bn_stats.

    Returns (rstd, nmean) [128, len(xts)] fp32 tiles.
    """
    n = len(xts)
    mv = pool.tile([P, n, 2], F32, tag="mv")
    for i, xt in enumerate(xts):
        st = pool.tile([P, 2, 6], F32, tag="bst")
        xr = xt.rearrange("p (s f) -> p s f", f=512)
        for s in range(2):
            nc.vector.bn_stats(st[:, s, :], xr[:, s, :])
        nc.vector.bn_aggr(mv[:, i, :], st[:])
    sd = pool.tile([P, n], F32, tag="sd")
    nc.scalar.activation(sd[:], mv[:, :, 1], AF.Sqrt, bias=eps_ap)
    rstd = pool.tile([P, n], F32, tag="rstd")
    nc.vector.reciprocal(rstd[:], sd[:])
    nmean = pool.tile([P, n], F32, tag="nmean")
    nc.vector.scalar_tensor_tensor(
        out=nmean[:], in0=mv[:, :, 0], scalar=-1.0, in1=rstd[:],
        op0=ALU.mult, op1=ALU.mult,
    )
    return rstd, nmean


def build_nc():
    nc = bacc.Bacc(None, target_bir_lowering=False)

    x = nc.declare_dram_parameter("x", [T, C], F32, isOutput=False)
    xq = nc.declare_dram_parameter("xq", [SR, C], F32, isOutput=False)
    wq = nc.declare_dram_parameter("wq", [C, C], BF16, isOutput=False)
    wk = nc.declare_dram_parameter("wk", [C, C], BF16, isOutput=False)
    wv = nc.declare_dram_parameter("wv", [C, C], BF16, isOutput=False)
    bq = nc.declare_dram_parameter("bq", [P, NHP], F32, isOutput=False)
    bv_row = nc.declare_dram_parameter("bv_row", [1, C], BF16, isOutput=False)
    wproj = nc.declare_dram_parameter("wproj", [C, C], BF16, isOutput=False)
    projb = nc.declare_dram_parameter("projb", [1, C], BF16, isOutput=False)
    lin1 = nc.declare_dram_parameter("lin1", [C, 4 * C], BF16, isOutput=False)
    blin1 = nc.declare_dram_parameter("blin1", [P, NF], F32, isOutput=False)
    lin2 = nc.declare_dram_parameter("lin2", [4 * C, C], BF16, isOutput=False)
    blin2_bc = nc.declare_dram_parameter("blin2_bc", [P, C], F32,
                                         isOutput=False)
    ident = nc.declare_dram_parameter("ident", [P, P], BF16, isOutput=False)
    masks = nc.declare_dram_parameter("masks", [P, 1024], BF16, isOutput=False)
    out = nc.declare_dram_parameter("out", [SR, C], F32, isOutput=True)

    x_tiles = x.rearrange("(n p) c -> n p c", p=P)
    xq_tiles = xq.rearrange("(n p) c -> n p c", p=P)
    out_tiles = out.rearrange("(n p) c -> n p c", p=P)

    with tile.TileContext(nc) as tc:
        with (
            tc.tile_pool(name="const", bufs=1) as const,
            tc.tile_pool(name="stat", bufs=3) as stat,
            tc.tile_pool(name="wB", bufs=3) as wB,    # [128,1024] bf16 h tiles
        ):
            id_sb = const.tile([P, P], BF16)
            nc.sync.dma_start(id_sb[:], ident[:])
            mask_sb = const.tile([P, 1024], BF16)
            nc.sync.dma_start(mask_sb[:], masks[:])
            bq_sb = const.tile([P, NHP], F32)
            nc.sync.dma_start(bq_sb[:], bq[:])
            bv_sb = const.tile([1, C], BF16)
            nc.sync.dma_start(bv_sb[:], bv_row[:])
            projb_sb = const.tile([1, C], BF16)
            nc.sync.dma_start(projb_sb[:], projb[:])
            blin1_sb = const.tile([P, NF], F32)
            nc.sync.dma_start(blin1_sb[:], blin1[:])
            blin2_sb = const.tile([P, C], F32)
            nc.sync.dma_start(blin2_sb[:], blin2_bc[:])
            ones64 = const.tile([1, HS], BF16)
            nc.vector.memset(ones64[:], 1.0)
            onescol = const.tile([1, P], BF16)
            nc.vector.memset(onescol[:], 1.0)
            eps_sb = const.tile([P, 1], F32)
            nc.vector.memset(eps_sb[:], EPS)

            # persistent across the whole kernel
            res = const.tile([P, NM * C], F32)      # xq, then residual
            oT = const.tile([P, NHP * SR], BF16)    # per-pair o^T blocks
            wp_sb = const.tile([P, NHP * C], BF16)  # proj weights

            with tc.tile_pool(name="abig", bufs=1) as abig:
                hT = abig.tile([P, NCt * T], BF16)    # h^T (c-tile j at j*T)
                hqT = abig.tile([P, NCt * SR], BF16)  # hq^T (my q rows)
                vaug = abig.tile([P, NT * VW], BF16)  # V+ones per t-tile

                # ones columns of vaug (col 64 of each head block)
                va4 = vaug[:].rearrange("p (t h e) -> p t h e", h=H, e=65)
                nc.vector.memset(va4[:, :, :, 64:65], 1.0)

                # ======= LN1 over full T -> hT; V proj per tile =======
                with (
                    tc.tile_pool(name="wv_p", bufs=1) as wv_p,
                    tc.tile_pool(name="wA", bufs=5) as wA,
                    tc.tile_pool(name="pT", bufs=2, space="PSUM") as pT,
                    tc.tile_pool(name="pV", bufs=2, space="PSUM") as pV,
                ):
                    # hq: LN1 on my q rows (same math as the full pass,
                    # duplicated on the per-core xq copy) -> hqT
                    for m in range(NM):
                        nc.sync.dma_start(
                            res[:, m * C:(m + 1) * C], xq_tiles[m]
                        )
                    rstd, nmean = _ln_group_stats(
                        nc, stat,
                        [res[:, m * C:(m + 1) * C] for m in range(NM)],
                        eps_sb[:],
                    )
                    for m in range(NM):
                        hqm = wB.tile([P, C], BF16, tag="ht")
                        nc.scalar.activation(
                            hqm[:], res[:, m * C:(m + 1) * C], AF.Identity,
                            bias=nmean[:, m:m + 1], scale=rstd[:, m:m + 1],
                        )
                        tp = pT.tile([P, C], BF16, tag="tp")
                        for j in range(NCt):
                            nc.tensor.transpose(
                                tp[:, j * P:(j + 1) * P],
                                hqm[:, j * P:(j + 1) * P], id_sb[:]
                            )
                        hqTm = hqT[:].rearrange(
                            "p (j s) -> p j s", j=NCt
                        )[:, :, m * P:(m + 1) * P]
                        nc.vector.tensor_copy(
                            hqTm,
                            tp[:].rearrange("p (j q) -> p j q", j=NCt),
                        )
                    wv_sb = wv_p.tile([P, NCt * C], BF16)
                    wv_t = wv.rearrange("(n p) e -> n p e", p=P)
                    for j in range(NCt):
                        nc.sync.dma_start(
                            wv_sb[:, j * C:(j + 1) * C], wv_t[j]
                        )

                    for grp in range(NT // 4):
                        xts = []
                        for i4 in range(4):
                            xt = wA.tile([P, C], F32, tag="xt")
                            nc.sync.dma_start(xt[:], x_tiles[grp * 4 + i4])
                            xts.append(xt)
                        rstd, nmean = _ln_group_stats(
                            nc, stat, [t[:] for t in xts], eps_sb[:],
                        )
                        for i4 in range(4):
                            i = grp * 4 + i4
                            ht = wB.tile([P, C], BF16, tag="ht")
                            nc.scalar.activation(
                                ht[:], xts[i4][:], AF.Identity,
                                bias=nmean[:, i4:i4 + 1],
                                scale=rstd[:, i4:i4 + 1],
                            )
                            tp = pT.tile([P, C], BF16, tag="tp")
                            for j in range(NCt):
                                nc.tensor.transpose(
                                    tp[:, j * P:(j + 1) * P],
                                    ht[:, j * P:(j + 1) * P], id_sb[:]
                                )
                            hTi = hT[:].rearrange(
                                "p (j t) -> p j t", j=NCt
                            )[:, :, i * P:(i + 1) * P]
                            nc.vector.tensor_copy(
                                hTi,
                                tp[:].rearrange("p (j q) -> p j q", j=NCt),
                            )
                            # V projection for this t-tile (16 heads)
                            for eh in range(2):
                                ps = pV.tile([P, 512], F32, tag="vps")
                                for j in range(NCt):
                                    nc.tensor.matmul(
                                        ps[:],
                                        hT[:, j * T + i * P:
                                           j * T + (i + 1) * P],
                                        wv_sb[:, j * C + eh * 512:
                                              j * C + (eh + 1) * 512],
                                        start=(j == 0), stop=False,
                                    )
                                nc.tensor.matmul(
                                    ps[:], onescol[:],
                                    bv_sb[:, eh * 512:(eh + 1) * 512],
                                    start=False, stop=True,
                                )
                                va = vaug[:, i * VW + eh * 8 * 65:
                                          i * VW + (eh + 1) * 8 * 65]
                                nc.scalar.activation(
                                    va.rearrange("p (h e) -> p h e", e=65)
                                    [:, :, 0:64],
                                    ps[:].rearrange("p (h e) -> p h e", e=64),
                                    AF.Copy,
                                )

                # proj weights: needed at proj time; stream during attention
                wp_t = wproj.rearrange("(n p) c -> n p c", p=P)
                for hp in range(NHP):
                    nc.sync.dma_start(
                        wp_sb[:, hp * C:(hp + 1) * C], wp_t[hp]
                    )

                # ============ attention: per head-pair ============
                wk_t = wk.rearrange("(n p) e -> n p e", p=P)
                wq_t = wq.rearrange("(n p) e -> n p e", p=P)
                with (
                    tc.tile_pool(name="wkq", bufs=2) as wkq,
                    tc.tile_pool(name="kTp", bufs=2) as kTp,
                    tc.tile_pool(name="qTp", bufs=2) as qTp,
                    tc.tile_pool(name="ptp", bufs=4) as ptp,
                    tc.tile_pool(name="pS", bufs=2, space="PSUM") as pS,
                    tc.tile_pool(name="pO", bufs=2, space="PSUM") as pO,
                    tc.tile_pool(name="pM", bufs=2, space="PSUM") as pM,
                ):
                    def emit_kq(hp):
                        """K^T/Q^T projection chains for pair hp.

                        Returns list of emit-thunks (5 chains) plus setup.
                        """
                        wk_sb = wkq.tile([P, NCt * P], BF16, tag="wk",
                                         name=f"wk{hp}")
                        wq_sb = wkq.tile([P, NCt * P], BF16, tag="wq",
                                         name=f"wq{hp}")
                        for j in range(NCt):
                            nc.sync.dma_start(
                                wk_sb[:, j * P:(j + 1) * P],
                                wk_t[j][:, hp * P:(hp + 1) * P],
                            )
                            nc.sync.dma_start(
                                wq_sb[:, j * P:(j + 1) * P],
                                wq_t[j][:, hp * P:(hp + 1) * P],
                            )
                        kT = kTp.tile([P, T], BF16, tag="kT",
                                      name=f"kT{hp}")
                        qT = qTp.tile([P, SR], BF16, tag="qT",
                                      name=f"qT{hp}")

                        def k_chain(tch):
                            def f():
                                ps = pM.tile([P, 512], F32, tag="mm",
                                             name=f"kc{hp}_{tch}")
                                for j in range(NCt):
                                    nc.tensor.matmul(
                                        ps[:],
                                        wk_sb[:, j * P:(j + 1) * P],
                                        hT[:, j * T + tch * 512:
                                           j * T + (tch + 1) * 512],
                                        start=(j == 0), stop=(j == NCt - 1),
                                    )
                                nc.vector.tensor_copy(
                                    kT[:, tch * 512:(tch + 1) * 512], ps[:]
                                )
                            return f

                        def q_chain():
                            ps = pM.tile([P, 512], F32, tag="mm",
                                         name=f"qc{hp}")
                            for j in range(NCt):
                                nc.tensor.matmul(
                                    ps[:],
                                    wq_sb[:, j * P:(j + 1) * P],
                                    hqT[:, j * SR:(j + 1) * SR],
                                    start=(j == 0), stop=(j == NCt - 1),
                                )
                            nc.vector.tensor_scalar_add(
                                qT[:], ps[:], bq_sb[:, hp:hp + 1]
                            )

                        thunks = [k_chain(t) for t in range(4)] + [q_chain]
                        return kT, qT, thunks

                    kT, qT, thunks = emit_kq(0)
                    for th in thunks:
                        th()

                    for hp in range(NHP):
                        if hp < NHP - 1:
                            kT_n, qT_n, thunks = emit_kq(hp + 1)
                        else:
                            thunks = []
                        opsA = pO.tile([65, SR], F32, tag="ops",
                                       name=f"opsA{hp}")
                        opsB = pO.tile([65, SR], F32, tag="ops",
                                       name=f"opsB{hp}")
                        ti = 0
                        for ch in range(2):
                            n_vis = 12 + 4 * ch
                            prev = None
                            for g in range(n_vis // 4):
                                sA = pS.tile([P, 1024], F32, tag="sc",
                                             name=f"sA{hp}_{ch}_{g}")
                                sB = pS.tile([P, 1024], F32, tag="sc",
                                             name=f"sB{hp}_{ch}_{g}")
                                for t4 in range(4):
                                    tt = g * 4 + t4
                                    nc.tensor.matmul(
                                        sA[:, t4 * 256:(t4 + 1) * 256],
                                        kT[0:64, tt * P:(tt + 1) * P],
                                        qT[0:64, ch * 256:(ch + 1) * 256],
                                        start=True, stop=True,
                                    )
                                    nc.tensor.matmul(
                                        sB[:, t4 * 256:(t4 + 1) * 256],
                                        kT[64:128, tt * P:(tt + 1) * P],
                                        qT[64:128, ch * 256:(ch + 1) * 256],
                                        start=True, stop=True,
                                    )
                                if ti < len(thunks):
                                    thunks[ti]()
                                    ti += 1
                                ptA = ptp.tile([P, 1024], BF16, tag="pt",
                                               name=f"pA{hp}_{ch}_{g}")
                                ptB = ptp.tile([P, 1024], BF16, tag="pt",
                                               name=f"pB{hp}_{ch}_{g}")
                                nc.scalar.activation(
                                    ptA[:], sA[:], AF.Exp, scale=ATT_SCALE
                                )
                                nc.scalar.activation(
                                    ptB[:], sB[:], AF.Exp, scale=ATT_SCALE
                                )
                                if g == 2 + ch:  # boundary group: masks
                                    nc.vector.tensor_mul(
                                        ptA[:], ptA[:], mask_sb[:]
                                    )
                                    nc.vector.tensor_mul(
                                        ptB[:], ptB[:], mask_sb[:]
                                    )

                                def attv(gg, pA, pB):
                                    for t4 in range(4):
                                        tt = gg * 4 + t4
                                        nc.tensor.matmul(
                                            opsA[:, ch * 256:(ch + 1) * 256],
                                            vaug[:, tt * VW + 2 * hp * 65:
                                                 tt * VW + 2 * hp * 65 + 65],
                                            pA[:, t4 * 256:(t4 + 1) * 256],
                                            start=(tt == 0),
                                            stop=(tt == n_vis - 1),
                                        )
                                        nc.tensor.matmul(
                                            opsB[:, ch * 256:(ch + 1) * 256],
                                            vaug[:,
                                                 tt * VW + (2 * hp + 1) * 65:
                                                 tt * VW + (2 * hp + 1) * 65
                                                 + 65],
                                            pB[:, t4 * 256:(t4 + 1) * 256],
                                            start=(tt == 0),
                                            stop=(tt == n_vis - 1),
                                        )

                                if prev is not None:
                                    attv(*prev)
                                prev = (g, ptA, ptB)
                            attv(*prev)

                        # normalize: oT[...] = ops[0:64] / ops[64]
                        for r, ops in ((0, opsA), (1, opsB)):
                            # 1/Z = exp(-ln Z): both fns live in the same
                            # ACT table set as the attention Exp
                            lz = stat.tile([1, SR], F32, tag="lz")
                            nc.scalar.activation(
                                lz[:], ops[64:65, :], AF.Ln
                            )
                            rd = stat.tile([1, SR], BF16, tag="rd")
                            nc.scalar.activation(
                                rd[:], lz[:], AF.Exp, scale=-1.0
                            )
                            rb = pM.tile([P, 512], F32, tag="mm",
                                         name=f"rb{hp}_{r}")
                            nc.tensor.matmul(
                                rb[0:64, :], ones64[:], rd[:],
                                start=True, stop=True,
                            )
                            rbs = stat.tile([64, SR], BF16, tag="rbs")
                            nc.vector.tensor_copy(rbs[:], rb[0:64, :])
                            nc.vector.tensor_mul(
                                oT[64 * r:64 * (r + 1),
                                   hp * SR:(hp + 1) * SR],
                                ops[0:64, :], rbs[:],
                            )
                        if hp < NHP - 1:
                            kT, qT = kT_n, qT_n

            # ============ proj + residual -> res; LN2 -> h2T; FFN =======
            with tc.tile_pool(name="post", bufs=1) as post:
                h2T = post.tile([P, NCt * SR], BF16)
                gT = post.tile([P, NF * SR], BF16)
                with (
                    tc.tile_pool(name="pP2", bufs=3, space="PSUM") as pP2,
                    tc.tile_pool(name="pT2", bufs=2, space="PSUM") as pT2,
                ):
                    for m in range(NM):
                        for nh in range(2):
                            ps = pP2.tile([P, 512], F32, tag="mm")
                            for hp in range(NHP):
                                nc.tensor.matmul(
                                    ps[:],
                                    oT[:, hp * SR + m * P:
                                       hp * SR + (m + 1) * P],
                                    wp_sb[:, hp * C + nh * 512:
                                          hp * C + (nh + 1) * 512],
                                    start=(hp == 0), stop=False,
                                )
                            # proj bias via K=1 matmul (ones x bias row)
                            nc.tensor.matmul(
                                ps[:], onescol[:],
                                projb_sb[:, nh * 512:(nh + 1) * 512],
                                start=False, stop=True,
                            )
                            rm = res[:, m * C + nh * 512:
                                     m * C + (nh + 1) * 512]
                            nc.vector.tensor_add(rm, rm, ps[:])
                    rstd, nmean = _ln_group_stats(
                        nc, stat,
                        [res[:, m * C:(m + 1) * C] for m in range(NM)],
                        eps_sb[:],
                    )
                    for m in range(NM):
                        h2 = wB.tile([P, C], BF16, tag="ht")
                        nc.scalar.activation(
                            h2[:], res[:, m * C:(m + 1) * C], AF.Identity,
                            bias=nmean[:, m:m + 1], scale=rstd[:, m:m + 1],
                        )
                        tp = pT2.tile([P, C], BF16, tag="tp")
                        for j in range(NCt):
                            nc.tensor.transpose(
                                tp[:, j * P:(j + 1) * P],
                                h2[:, j * P:(j + 1) * P], id_sb[:]
                            )
                        h2Tm = h2T[:].rearrange(
                            "p (j s) -> p j s", j=NCt
                        )[:, :, m * P:(m + 1) * P]
                        nc.vector.tensor_copy(
                            h2Tm,
                            tp[:].rearrange("p (j q) -> p j q", j=NCt),
                        )
                        # fold lin2 bias into res for the final add
                        nc.vector.tensor_add(
                            res[:, m * C:(m + 1) * C],
                            res[:, m * C:(m + 1) * C], blin2_sb[:],
                        )

                # ============ FFN1 (gelu) -> gT ============
                l1_t = lin1.rearrange("(n p) f -> n p f", p=P)
                with (
                    tc.tile_pool(name="ffB", bufs=2) as ffB,
                    tc.tile_pool(name="pG", bufs=2, space="PSUM") as pG,
                ):
                    for quar in range(4):
                        l1h = ffB.tile([P, NCt * C], BF16, tag="l1")
                        for j in range(NCt):
                            nc.sync.dma_start(
                                l1h[:, j * C:(j + 1) * C],
                                l1_t[j][:, quar * C:(quar + 1) * C],
                            )
                        for fl in range(NF // 4):
                            ft = quar * (NF // 4) + fl
                            ps = pG.tile([P, 512], F32, tag="mm")
                            for j in range(NCt):
                                nc.tensor.matmul(
                                    ps[:],
                                    l1h[:, j * C + fl * P:
                                        j * C + (fl + 1) * P],
                                    h2T[:, j * SR:(j + 1) * SR],
                                    start=(j == 0), stop=(j == NCt - 1),
                                )
                            nc.scalar.activation(
                                gT[:, ft * SR:(ft + 1) * SR], ps[:], AF.Gelu,
                                bias=blin1_sb[:, ft:ft + 1],
                            )

                # ============ FFN2 + residual -> out ============
                l2_t = lin2.rearrange("(n p) c -> n p c", p=P)
                with (
                    tc.tile_pool(name="ffW", bufs=3) as ffW,
                    tc.tile_pool(name="pF", bufs=1, space="PSUM") as pF,
                ):
                    for nh in range(2):
                        fps = []
                        for m in range(NM):
                            fpt = pF.tile([P, 512], F32, tag=f"ff{m}")
                            fps.append(fpt)
                        for ft in range(NF):
                            l2 = ffW.tile([P, 512], BF16, tag="l2")
                            nc.sync.dma_start(
                                l2[:], l2_t[ft][:, nh * 512:(nh + 1) * 512]
                            )
                            for m in range(NM):
                                nc.tensor.matmul(
                                    fps[m][:],
                                    gT[:, ft * SR + m * P:
                                       ft * SR + (m + 1) * P],
                                    l2[:],
                                    start=(ft == 0), stop=(ft == NF - 1),
                                )
                        for m in range(NM):
                            o_sb = ffW.tile([P, 512], F32, tag="osb")
                            nc.vector.tensor_add(
                                o_sb[:], fps[m][:],
                                res[:, m * C + nh * 512:
                                    m * C + (nh + 1) * 512],
                            )
                            nc.sync.dma_start(
                                out_tiles[m][:, nh * 512:(nh + 1) * 512],
                                o_sb[:],
                            )

    nc.compile()
    return nc


_NC = None


def _get_nc():
    global _NC
    if _NC is None:
        _NC = build_nc()
    return _NC


def kernel(**inputs):
    nc = _get_nc()
    bf = ml_dtypes.bfloat16
    f32 = np.float32

    x = np.asarray(inputs["x"], f32)
    Wq = np.asarray(inputs["Wq"], f32)
    Wk = np.asarray(inputs["Wk"], f32)
    Wv = np.asarray(inputs["Wv"], f32)
    bq = np.asarray(inputs["bq"], f32)
    bv = np.asarray(inputs["bv"], f32)
    proj_w = np.asarray(inputs["proj_w"], f32)
    proj_b = np.asarray(inputs["proj_b"], f32)
    ln1_w = np.asarray(inputs["ln1_w"], f32)
    ln1_b = np.asarray(inputs["ln1_b"], f32)
    ln2_w = np.asarray(inputs["ln2_w"], f32)
    ln2_b = np.asarray(inputs["ln2_b"], f32)
    lin1_w = np.asarray(inputs["lin1_w"], f32)
    lin1_b = np.asarray(inputs["lin1_b"], f32)
    lin2_w = np.asarray(inputs["lin2_w"], f32)
    lin2_b = np.asarray(inputs["lin2_b"], f32)

    # LN1 gamma folded into QKV weights; beta folded into biases.
    Wq_f = Wq * ln1_w[None, :, None]              # [H, C, HS]
    Wk_f = Wk * ln1_w[None, :, None]
    Wv_f = Wv * ln1_w[None, :, None]
    bq_eff = bq + np.einsum("c,hcd->hd", ln1_b, Wq)   # [H, HS]
    bv_eff = bv + np.einsum("c,hcd->hd", ln1_b, Wv)
    # (bk dropped: softmax shift invariance)
    wq_full = np.ascontiguousarray(
        Wq_f.transpose(1, 0, 2).reshape(C, C)).astype(bf)
    wk_full = np.ascontiguousarray(
        Wk_f.transpose(1, 0, 2).reshape(C, C)).astype(bf)
    wv_full = np.ascontiguousarray(
        Wv_f.transpose(1, 0, 2).reshape(C, C)).astype(bf)
    bq_t = np.ascontiguousarray(bq_eff.reshape(NHP, P).T).astype(f32)
    bv_rowh = bv_eff.reshape(1, C).astype(bf)

    # LN2 gamma folded into lin1; beta into its bias.
    lin1_f = (lin1_w * ln2_w[:, None]).astype(bf)
    blin1_eff = lin1_b + ln2_b @ lin1_w
    blin1_t = np.ascontiguousarray(blin1_eff.reshape(NF, P).T).astype(f32)
    lin2_bf = lin2_w.astype(bf)
    blin2_bc = np.ascontiguousarray(
        np.broadcast_to(lin2_b, (P, C))).astype(f32)

    proj_bf = proj_w.astype(bf)
    projb_row = proj_b.reshape(1, C).astype(bf)
    ident = np.eye(P, dtype=bf)

    in_maps = []
    for c in range(8):
        b, g = divmod(c, 2)
        # my q rows: 64-row blocks {2j+g}, local row 64j+r
        j = np.arange(NCt)
        r = np.arange(64)
        rows = (T - CUT) + 64 * (2 * j[:, None] + g) + r[None, :]
        rows = rows.reshape(-1)
        xq_c = np.ascontiguousarray(x[b][rows]).astype(f32)
        # masks: [128, 4*256]: block i, col 64*jj+rr:
        #   visible iff toff <= 128*jj + 64*g + rr - 128*i
        toff = np.arange(P)[:, None]
        i_b = np.arange(4)[:, None, None]
        jj = np.arange(4)[None, :, None]
        rr = np.arange(64)[None, None, :]
        thr = (128 * jj + 64 * g + rr - 128 * i_b).reshape(1, 1024)
        mask_c = (toff <= thr).astype(bf)
        in_maps.append({
            "x": np.ascontiguousarray(x[b]),
            "xq": xq_c,
            "wq": wq_full, "wk": wk_full, "wv": wv_full,
            "bq": bq_t, "bv_row": bv_rowh,
            "wproj": proj_bf, "projb": projb_row,
            "lin1": lin1_f, "blin1": blin1_t,
            "lin2": lin2_bf, "blin2_bc": blin2_bc,
            "ident": ident, "masks": np.ascontiguousarray(mask_c),
        })

    resl = run_bass_kernel_spmd(nc, in_maps, core_ids=list(range(8)))
    out_full = np.empty((B, CUT, C), f32)
    jj = np.arange(NCt)
    rr = np.arange(64)
    for c in range(8):
        b, g = divmod(c, 2)
        rows = (64 * (2 * jj[:, None] + g) + rr[None, :]).reshape(-1)
        out_full[b, rows, :] = resl.results[c]["out"]
    return out_full
